# revision 1
# baseline (speedup 1.0000x reference)
# Fused dynamic-conv (CondInst-style) + dice loss kernel for 8x TRN2 NeuronCores.
#
# Reference computation (per batch image b, object o):
#   weight[b,o,:] = conv_weight[b, :, ind[b,o]]           (gather, 593 params)
#   feat = concat(seg_feat[b], x_rel(o), y_rel(o))        ([18, 128*128])
#   h1 = relu(w1 @ feat + b1); h2 = relu(w2 @ h1 + b2)    (16-ch dynamic 1x1 convs)
#   out = sigmoid(w3 . h2 + b3)                           ([128*128])
#   dice over masked objects -> scalar loss
#
# Strategy:
#  * Host gathers the 593 dynamic params per object (303KB of the 155MB
#    conv_weight) and packs active (mask=1) objects into groups of 8.
#    x_rel/y_rel are separable: x_rel = xg/128 - x_o/128, so the grid rows are
#    object-independent and the object offsets fold into an effective bias
#    b1_eff. One shared [18, HW] feature map per image serves all its objects.
#  * A group of 8 objects => block-diagonal weights, so the three GEMMs run as
#    [K<=128, M<=128, N=512] fp16 matmuls at full PE column rate.
#  * Work unit = (group, half-image) = 8192 px. Units are distributed over the
#    8 cores (SPMD single program; all data dependence lives in the inputs).
#  * gemm3 ([K=128, M=32, N=512], fp16, column-tiled) writes quadrant-aligned
#    blocks of a [128, 2048] PSUM region; lhsT3 columns 8:32 are zero so
#    filler rows are exact zeros and a -50 bias drives them to sigmoid ~= 0.
#  * Dice partials per unit via fused ops: scalar_tensor_tensor (sum pred*tgt)
#    and Square activation with accum_out (sum pred^2); the host does the
#    final tiny reduction plus sum(tgt^2), which is network-independent.
import numpy as np
from contextlib import ExitStack

import concourse.bass as bass
import concourse.tile as tile
from concourse import mybir, bacc
from concourse.bass_utils import run_bass_kernel_spmd

C = 16
WT = 593
B, O, H, W = 4, 32, 128, 128
HW = H * W
N_CORES = 8
GRP = 8            # objects per block-diagonal group
HALF = 8192        # pixels per work unit
NT = HALF // 512   # 512-px moving-tiles per unit (16)

F32 = mybir.dt.float32
F32R = mybir.dt.float32r
F16 = mybir.dt.float16
ACTF = mybir.ActivationFunctionType
ALU = mybir.AluOpType

# wpack free-dim layout (per unit, [128, 96] float16). gemm1/gemm2 run as 4
# concurrent diagonal 32x32 PE tiles (row band r = objects 2r, 2r+1), so their
# weights live per 32-row band:
#   0:32   lhsT3 (block-diag w3; cols 8:32 zero)
#   32:64  lhsT2 band blocks: rows 32r hold diag(w2[2r]^T, w2[2r+1]^T)
#   64:96  lhsT1 band blocks: rows 32r+0:18 hold w1^T of objects 2r, 2r+1
# bias layout (per unit, [128, 3] float32): 0 = b1_eff, 1 = b2, 2 = b3/-50
WCOLS = 96


def host_pack(seg_feat, conv_weight, mask, ind, target):
    cw = conv_weight.reshape(B, WT, HW)
    weight = np.take_along_axis(cw, ind[:, None, :].astype(np.int64), axis=2)
    weight = np.ascontiguousarray(weight.transpose(0, 2, 1))  # [B, O, WT]
    s0 = (C + 2) * C
    w1 = weight[..., :s0].reshape(B, O, C, C + 2)
    b1 = weight[..., s0:s0 + C]
    w2 = weight[..., s0 + C:s0 + C + C * C].reshape(B, O, C, C)
    b2 = weight[..., s0 + C + C * C:s0 + 2 * C + C * C]
    w3 = weight[..., s0 + 2 * C + C * C:s0 + 3 * C + C * C]
    b3 = weight[..., -1]
    xo = (ind % W).astype(np.float32)
    yo = (ind // W).astype(np.float32)

    units = []  # (b, objs[8 padded with -1], half)
    for b in range(B):
        objs = [o for o in range(O) if mask[b, o] == 1]
        for g0 in range(0, len(objs), GRP):
            grp = objs[g0:g0 + GRP]
            grp = grp + [-1] * (GRP - len(grp))
            for half in range(2):
                units.append((b, grp, half))
    per_core = [[] for _ in range(N_CORES)]
    for i, u in enumerate(units):
        per_core[i % N_CORES].append(u)
    NG = max(1, max(len(u) for u in per_core))
    for ci in range(N_CORES):
        while len(per_core[ci]) < NG:
            per_core[ci].append((0, [-1] * GRP, 0))

    px = np.arange(HW, dtype=np.float32)
    xg = (px % W) / 128.0
    yg = np.floor(px / W) / 128.0
    tgt_flat = target.reshape(B, O, HW)

    in_maps = []
    for ci in range(N_CORES):
        feat_pack = np.zeros((NG, 18, HALF), np.float16)
        wpack = np.zeros((NG, 128, WCOLS), np.float16)
        bias_pack = np.zeros((NG, 128, 3), np.float32)
        tgt_pack = np.zeros((NG, 128, 2048), np.float16)
        for u, (b, grp, half) in enumerate(per_core[ci]):
            sl = slice(half * HALF, (half + 1) * HALF)
            feat_pack[u, :16] = seg_feat[b].reshape(C, HW)[:, sl]
            feat_pack[u, 16] = xg[sl]
            feat_pack[u, 17] = yg[sl]
            bias_pack[u, :, 2] = -50.0  # filler-row sigmoid bias
            for oo, o in enumerate(grp):
                if o < 0:
                    continue
                r, p = oo // 2, oo % 2
                wpack[u, 32 * r:32 * r + 18, 64 + 16 * p:64 + 16 * p + 16] = \
                    w1[b, o].T.astype(np.float16)
                b1e = (b1[b, o] - w1[b, o, :, 16] * (xo[b, o] / 128.0)
                       - w1[b, o, :, 17] * (yo[b, o] / 128.0))
                bias_pack[u, 16 * oo:16 * oo + 16, 0] = b1e
                wpack[u, 32 * r + 16 * p:32 * r + 16 * p + 16,
                      32 + 16 * p:32 + 16 * p + 16] = \
                    w2[b, o].T.astype(np.float16)
                bias_pack[u, 16 * oo:16 * oo + 16, 1] = b2[b, o]
                wpack[u, 16 * oo:16 * oo + 16, oo] = \
                    w3[b, o].astype(np.float16)
                for q in range(4):
                    bias_pack[u, 32 * q + oo, 2] = b3[b, o]
                # tgt in the packed sigmoid layout: partition 32q+oo holds
                # moving-tiles t = 4k+q at free cols 512k..512k+512
                for t in range(NT):
                    q, k = t % 4, t // 4
                    g0 = half * HALF + t * 512
                    tgt_pack[u, 32 * q + oo, 512 * k:512 * k + 512] = \
                        tgt_flat[b, o, g0:g0 + 512].astype(np.float16)
        in_maps.append({"feat": feat_pack, "wpack": wpack,
                        "bias": bias_pack, "tgt": tgt_pack})
    return in_maps, per_core, NG


_PROGRAM_CACHE = {}


def build_program(NG):
    if NG in _PROGRAM_CACHE:
        return _PROGRAM_CACHE[NG]
    nc = bacc.Bacc("TRN2", target_bir_lowering=False, debug=False,
                   enable_asserts=False, num_devices=N_CORES)
    feat_t = nc.dram_tensor("feat", (NG, 18, HALF), F16, kind="ExternalInput")
    wpack_t = nc.dram_tensor("wpack", (NG, 128, WCOLS), F16, kind="ExternalInput")
    bias_t = nc.dram_tensor("bias", (NG, 128, 3), F32, kind="ExternalInput")
    tgt_t = nc.dram_tensor("tgt", (NG, 128, 2048), F16, kind="ExternalInput")
    acc_t = nc.dram_tensor("acc", (4, 128, NG), F32, kind="ExternalOutput")

    with tile.TileContext(nc) as tc, ExitStack() as ctx:
        wpool = ctx.enter_context(tc.tile_pool(name="wpool", bufs=2))
        fpool = ctx.enter_context(tc.tile_pool(name="fpool", bufs=2))
        h1pool = ctx.enter_context(tc.tile_pool(name="h1pool", bufs=2))
        h2pool = ctx.enter_context(tc.tile_pool(name="h2pool", bufs=2))
        tpool = ctx.enter_context(tc.tile_pool(name="tpool", bufs=2))
        ppool = ctx.enter_context(tc.tile_pool(name="ppool", bufs=2))
        spool = ctx.enter_context(tc.tile_pool(name="spool", bufs=2))
        apool = ctx.enter_context(tc.tile_pool(name="apool", bufs=1))
        ps = ctx.enter_context(tc.tile_pool(name="ps", bufs=4, space="PSUM"))

        inter_acc = apool.tile([128, NG], F32)
        psq_acc = apool.tile([128, NG], F32)
        inter_acc2 = apool.tile([128, NG], F32)
        psq_acc2 = apool.tile([128, NG], F32)

        # Warm-up during the initial DMA wait: load the sigmoid table set
        # (covers relu/square/sigmoid, so no mid-kernel table switch) and run
        # dummy matmuls so the PE HAM clock-gate opens before real work.
        scr = apool.tile([128, 512], F16)
        nc.vector.memset(scr, 0.125)
        scr1 = apool.tile([128, 1], F32)
        nc.scalar.activation(scr1, scr[:, 0:1], ACTF.Sigmoid, bias=0.0, scale=1.0)
        pw = ps.tile([128, 1024], F32, tag="ps")
        for _ in range(4):
            nc.tensor.matmul(pw[:, 0:512], scr[:, 0:128], scr, start=True, stop=True)

        for u in range(NG):
            wt = wpool.tile([128, WCOLS], F16, tag="w")
            nc.sync.dma_start(out=wt, in_=wpack_t.ap()[u])
            bt = wpool.tile([128, 3], F32, tag="b")
            nc.gpsimd.dma_start(out=bt, in_=bias_t.ap()[u])
            ft = fpool.tile([114, HALF], F16, tag="f")
            for half2 in range(2):
                fsl = slice(4096 * half2, 4096 * half2 + 4096)
                for r in range(4):
                    nc.sync.dma_start(out=ft[32 * r:32 * r + 18, fsl],
                                      in_=feat_t.ap()[u][:, fsl])
            tg = tpool.tile([128, 2048], F16, tag="t")
            nc.gpsimd.dma_start(out=tg, in_=tgt_t.ap()[u])

            w3t = wt[:, 0:32]
            b1ap = bt[:, 0:1]
            b2ap = bt[:, 1:2]
            b3ap = bt[:, 2:3]

            h1 = h1pool.tile([128, HALF], F16, tag="h1")
            h2 = h2pool.tile([128, HALF], F16, tag="h2")

            # phase A: gemm1 + relu1(+bias); evacuations alternate ACT/DVE.
            # 4 PSUM slots of [128,1024] keep the PE streaming without gaps
            # (so the HAM clock-gate opens and stays open).
            for j in range(8):
                pa = ps.tile([128, 1024], F32, tag="ps")
                for s in range(2):
                    t = 2 * j + s
                    for r in range(4):
                        nc.tensor.matmul(
                            pa[32 * r:32 * r + 32, 512 * s:512 * s + 512],
                            wt[32 * r:32 * r + 18, 64:96],
                            ft[32 * r:32 * r + 18, 512 * t:512 * t + 512],
                            start=True, stop=True, tile_position=(32 * r, 32 * r))
                dst = h1[:, 1024 * j:1024 * j + 1024]
                if j % 2 == 0:
                    nc.scalar.activation(dst, pa, ACTF.Relu, bias=b1ap, scale=1.0)
                else:
                    nc.vector.tensor_scalar(out=dst, in0=pa, scalar1=b1ap,
                                            scalar2=0.0, op0=ALU.add, op1=ALU.max)
            # phase B: gemm2 + relu2(+bias), h2 in fp16
            for j in range(8):
                pb = ps.tile([128, 1024], F32, tag="ps")
                for s in range(2):
                    t = 2 * j + s
                    for r in range(4):
                        nc.tensor.matmul(
                            pb[32 * r:32 * r + 32, 512 * s:512 * s + 512],
                            wt[32 * r:32 * r + 32, 32:64],
                            h1[32 * r:32 * r + 32, 512 * t:512 * t + 512],
                            start=True, stop=True, tile_position=(32 * r, 32 * r))
                dst = h2[:, 1024 * j:1024 * j + 1024]
                if j % 2 == 1:
                    nc.scalar.activation(dst, pb, ACTF.Relu, bias=b2ap, scale=1.0)
                else:
                    nc.vector.tensor_scalar(out=dst, in0=pb, scalar1=b2ap,
                                            scalar2=0.0, op0=ALU.add, op1=ALU.max)
            # phase C: gemm3 column-tiled (fp16), quadrant-packed into two
            # [128, 1024] PSUM regions (tiles t=4k+q at partition 32q, col 512k)
            for half in range(2):
                pc = ps.tile([128, 1024], F32, tag="ps")
                for t2 in range(8):
                    q, k2 = t2 % 4, t2 // 4
                    t = 4 * (2 * half + k2) + q
                    nc.tensor.matmul(
                        pc[32 * q:32 * q + 32, 512 * k2:512 * k2 + 512],
                        w3t, h2[:, 512 * t:512 * t + 512],
                        start=True, stop=True, tile_position=(0, 32 * q))
                pred = ppool.tile([128, 1024], F32, tag="p")
                nc.scalar.activation(pred, pc, ACTF.Sigmoid, bias=b3ap, scale=1.0)
                tgs = tg[:, 1024 * half:1024 * half + 1024]
                prod = spool.tile([128, 1024], F32, tag="s")
                nc.vector.scalar_tensor_tensor(
                    out=prod, in0=pred, scalar=0.0, in1=tgs,
                    op0=ALU.add, op1=ALU.mult,
                    accum_out=inter_acc[:, u:u + 1] if half == 0 else inter_acc2[:, u:u + 1])
                sq = spool.tile([128, 1024], F32, tag="s")
                nc.scalar.activation(
                    sq, pred, ACTF.Square,
                    accum_out=psq_acc[:, u:u + 1] if half == 0 else psq_acc2[:, u:u + 1])

        nc.sync.dma_start(out=acc_t.ap()[0], in_=inter_acc)
        nc.sync.dma_start(out=acc_t.ap()[1], in_=psq_acc)
        nc.sync.dma_start(out=acc_t.ap()[2], in_=inter_acc2)
        nc.sync.dma_start(out=acc_t.ap()[3], in_=psq_acc2)

    nc.compile()
    _PROGRAM_CACHE[NG] = nc
    return nc


def _run(inputs, trace=False):
    seg_feat = np.asarray(inputs["seg_feat"], np.float32)
    conv_weight = np.asarray(inputs["conv_weight"], np.float32)
    mask = np.asarray(inputs["mask"])
    ind = np.asarray(inputs["ind"])
    target = np.asarray(inputs["target"], np.float32)

    in_maps, per_core, NG = host_pack(seg_feat, conv_weight, mask, ind, target)
    nc = build_program(NG)
    res = run_bass_kernel_spmd(nc, in_maps, core_ids=list(range(N_CORES)),
                               trace=trace)

    inter = np.zeros(B, np.float64)
    predsq = np.zeros(B, np.float64)
    for ci in range(N_CORES):
        acc = res.results[ci]["acc"]
        for u, (b, grp, half) in enumerate(per_core[ci]):
            if all(o < 0 for o in grp):
                continue
            inter[b] += acc[0, :, u].sum(dtype=np.float64)
            predsq[b] += acc[1, :, u].sum(dtype=np.float64)
            inter[b] += acc[2, :, u].sum(dtype=np.float64)
            predsq[b] += acc[3, :, u].sum(dtype=np.float64)
    tgtsq = ((target.reshape(B, O, HW).astype(np.float64) ** 2)
             * mask[:, :, None]).sum(axis=(1, 2))
    loss = 1.0 - (2.0 * inter + 1.0) / (predsq + tgtsq + 1.0)
    return np.float32(loss.mean()), res


def kernel(**inputs):
    loss, _ = _run(inputs, trace=False)
    return np.array(loss, dtype=np.float32)

